# revision 20
# baseline (speedup 1.0000x reference)
"""Trainium2 Bass kernel for NeuroISNet GNN message passing.

Strategy (8 NeuronCores, one trn2 chip):
  - Batch b -> core pair (2b, 2b+1); each core owns 2048 of 4096 node rows.
  - The dominant einsum msg = x @ m runs in fp8e4 with DoubleRow perf
    mode; x^T stays resident in SBUF as fp8 (8MB), with its contraction
    rows HOST-PERMUTED per core to [own 2048 | partner 2048].
  - Own-half m chunks feed the bmm DIRECTLY from SBUF (mloc tiles, no
    collective round-trip).  Only the partner half moves: a masked
    ReduceScatter (per-core 0/1 mask inputs keep the SPMD program
    uniform: each core stages [m*rho | m*(1-rho)]; the add-reduce then
    delivers exactly the partner's m at a fixed address).  The bmm
    consumes [own 8 pairs | RS0 4 pairs | RS1 4 pairs] so both
    collectives hide completely under guaranteed-local PE work, keeping
    the PE HAM clock-gate at full rate.
  - Iteration 1 is rank-1 (identical initial rows): h1, c1 and
    m1 = MLP(LN(h1)) are computed on the HOST in f32 and shipped as
    inputs; the device starts at iteration 2 with a full m1.
  - LayerNorm scale 1/sqrt(var+eps) is computed entirely on the vector
    engine (bitcast + magic constant + 1 Newton step) and applied as a
    DVE tensor_scalar.  Every scalar-engine function stays inside one
    activation table set (sigmoid/tanh/relu/copy).
  - LSTM elementwise math in bf16 (2x DVE throughput); an explicit slot
    schedule staggers per-rb LSTM/LN/MLP chains (Act/DVE) under the
    in-order PE stream.
"""

import numpy as np
import ml_dtypes

import concourse.bass as bass
import concourse.mybir as mybir
import concourse.tile as tile
from concourse import bacc
from concourse.bass_utils import run_bass_kernel_spmd

BF = ml_dtypes.bfloat16
E4 = ml_dtypes.float8_e4m3
bf16 = mybir.dt.bfloat16
fp8 = mybir.dt.float8e4
f32 = mybir.dt.float32
i32 = mybir.dt.int32

B, N, H, ITERS = 4, 4096, 128, 8
EPS = 1e-5
NCORES = 8
R = N // 2              # rows per core
GROUPS = [[0, 1], [2, 3], [4, 5], [6, 7]]
MAGIC = 0x5F3759DF

AF = mybir.ActivationFunctionType
ALU = mybir.AluOpType
DR = mybir.MatmulPerfMode.DoubleRow


def build_module(n_nodes=N, iters=ITERS):
    r = n_nodes // 2            # local rows per core
    kc = n_nodes // 128         # k-chunks (global)
    npair = kc // 2             # DoubleRow chunk pairs (16)
    nown = npair // 2           # own-half pairs (8)
    nrb = max(1, r // 512)      # local 512-row blocks
    rbsz = r // nrb             # 512
    ntt = rbsz // 128           # 128-col tiles per rb (4)
    W = 2 * ntt * H             # free width of one RS rank block (1024)

    nc = bacc.Bacc("TRN2", target_bir_lowering=False, debug=False,
                   num_devices=NCORES)

    din = lambda name, shape, dt: nc.dram_tensor(name, shape, dt,
                                                 kind="ExternalInput")
    xt_in = din("xt", [n_nodes, r], fp8)          # row-permuted: own|remote
    h1_in = din("h1", [H, r], bf16)
    c1_in = din("c1", [H, r], bf16)
    m1q_in = din("m1q", [128, kc, H], fp8)        # full m1, permuted chunks
    rs_in = din("rs", [1, r], bf16)
    b3r_in = din("b3r", [1, H], bf16)
    w1gt_in = din("w1gt", [H, H], bf16)
    w2t_in = din("w2t", [H, H], bf16)
    w3t_in = din("w3t", [H, H], bf16)
    vw1t_in = din("vw1gt", [H, H], bf16)
    vw2t_in = din("vw2t", [H, H], bf16)
    vw3t_in = din("vw3t", [H, 1], bf16)
    wgq_in = din("wgq", [H, 2, 4 * H], fp8)
    b1c_in = din("b1c", [H, 1], f32)
    b2c_in = din("b2c", [H, 1], f32)
    vb1c_in = din("vb1c", [H, 1], f32)
    vb2c_in = din("vb2c", [H, 1], f32)
    bgc_in = din("bgc", [H, 4], f32)
    ident_in = din("ident", [H, H], bf16)
    selm_in = din("selm", [H, 2 * (r // max(1, r // 512)) // 128 * H // 4], i32)

    votes_out = nc.dram_tensor("votes", [1, r], f32, kind="ExternalOutput")

    with tile.TileContext(nc) as tc:
        with tc.tile_pool(name="const", bufs=1) as cp, \
             tc.tile_pool(name="state", bufs=1) as st, \
             tc.tile_pool(name="work", bufs=1) as wk, \
             tc.tile_pool(name="ps", bufs=1, space="PSUM") as ps, \
             tc.tile_pool(name="dram", bufs=1, space="DRAM") as dr:

            # ---- constants ----
            def cload(inp, shape, dt, tag):
                t = cp.tile(shape, dt, tag=tag, name=tag)
                nc.sync.dma_start(t[:], inp[:])
                return t

            w1gt = cload(w1gt_in, [H, H], bf16, "w1gt")
            w2t = cload(w2t_in, [H, H], bf16, "w2t")
            w3t = cload(w3t_in, [H, H], bf16, "w3t")
            vw1gt = cload(vw1t_in, [H, H], bf16, "vw1gt")
            vw2t = cload(vw2t_in, [H, H], bf16, "vw2t")
            vw3t = cload(vw3t_in, [H, 1], bf16, "vw3t")
            wgq = cload(wgq_in, [H, 2, 4 * H], fp8, "wgq")
            b1c = cload(b1c_in, [H, 1], f32, "b1c")
            b2c = cload(b2c_in, [H, 1], f32, "b2c")
            vb1c = cload(vb1c_in, [H, 1], f32, "vb1c")
            vb2c = cload(vb2c_in, [H, 1], f32, "vb2c")
            bgc = cload(bgc_in, [H, 4], f32, "bgc")
            ident = cload(ident_in, [H, H], bf16, "ident")
            rs_sb = cload(rs_in, [1, r], bf16, "rs")
            b3r = cload(b3r_in, [1, H], bf16, "b3r")
            selm = cload(selm_in, [H, 2 * ntt * H // 4], i32, "selm")
            m1q = cload(m1q_in, [128, kc, H], fp8, "m1q")
            mgc = cp.tile([128, ntt], i32, tag="mgc", name="mgc")
            nc.vector.memset(mgc[:], MAGIC)

            # ---- state tiles ----
            h_rb, c_rb = [], []
            for rb in range(nrb):
                ht = st.tile([H, rbsz], bf16, tag=f"h{rb}", name=f"h{rb}")
                nc.sync.dma_start(ht[:], h1_in[:, rb * rbsz:(rb + 1) * rbsz])
                ct = st.tile([H, rbsz], bf16, tag=f"c{rb}", name=f"c{rb}")
                nc.sync.dma_start(ct[:], c1_in[:, rb * rbsz:(rb + 1) * rbsz])
                h_rb.append(ht)
                c_rb.append(ct)
            hnL = st.tile([128, r], fp8, tag="hnL", name="hnL")
            # packed gates moving operand: plane0 = msg (fp8), plane1 = h
            gh_rb = []
            for rb in range(nrb):
                ght = st.tile([128, 2, rbsz], fp8, tag=f"gh{rb}",
                              name=f"gh{rb}")
                nc.vector.tensor_copy(ght[:, 1, :], h_rb[rb][:])
                gh_rb.append(ght)

            # ---- resident x^T chunk pairs (program order: own|remote) ----
            xt = [st.tile([128, 2, r], fp8, tag=f"xt{p}", name=f"xt{p}")
                  for p in range(npair)]
            for p in range(npair):
                for i in range(2):
                    nc.sync.dma_start(
                        xt[p][:, i, :],
                        xt_in[(2 * p + i) * 128:(2 * p + i + 1) * 128, :])

            # ---- DRAM bounce buffers for AllGather (groups: rb{0,1}, {2}, {3}) ----
            GW = [2 * ntt * H, ntt * H, ntt * H]
            cc_in = [dr.tile([128, GW[g]], fp8, tag=f"cci{g}", bufs=2,
                             name=f"cci{g}") for g in range(3)]
            cc_out = [dr.tile([2 * 128, GW[g]], fp8, tag=f"cco{g}", bufs=2,
                              name=f"cco{g}") for g in range(3)]

            def bmm_own_start(rb, it):
                """open the accumulator with the b3 rank-1 term."""
                mp = ps.tile([H, rbsz], f32, tag="pacc", bufs=4,
                             name=f"msg_{it}_{rb}")
                sl = slice(rb * rbsz, (rb + 1) * rbsz)
                nc.tensor.matmul(mp[:], b3r[:], rs_sb[:, sl],
                                 start=True, stop=False)
                return mp

            def bmm_own_frag(rb, mp, it, k, mloc_k):
                """own pairs (2k, 2k+1) -- issued as soon as mloc_k exists."""
                sl = slice(rb * rbsz, (rb + 1) * rbsz)
                for i in range(2):
                    q = 2 * k + i
                    stat = (m1q[:, 2 * q:2 * q + 2, :] if it == 2
                            else mloc_k[:, 2 * i:2 * i + 2, :])
                    nc.tensor.matmul(mp[:], stat, xt[q][:, :, sl],
                                     start=False, stop=False, perf_mode=DR)

            REM_Q0 = [nown, nown + 4, nown + 6]
            REM_QN = [4, 2, 2]

            def bmm_rem_rb(rb, mp, it, mR, half, stop):
                """remote pairs from AG group `half` (+ stop on last)."""
                sl = slice(rb * rbsz, (rb + 1) * rbsz)
                q0 = REM_Q0[half]
                qn = REM_QN[half]
                for q in range(q0, q0 + qn):
                    if it == 2:
                        stat = m1q[:, 2 * q:2 * q + 2, :]
                    else:
                        i = q - q0
                        stat = mR[:, 2 * i:2 * i + 2, :]
                    nc.tensor.matmul(mp[:], stat, xt[q][:, :, sl],
                                     start=False, stop=(stop and q == q0 + qn - 1),
                                     perf_mode=DR)

            def drain_rb(rb, mp, it):
                """drain the bmm PSUM accumulator into the gates operand."""
                if rb % 2 == 0:
                    nc.scalar.activation(gh_rb[rb][:, 0, :], mp[:], AF.Copy)
                else:
                    nc.vector.tensor_copy(gh_rb[rb][:, 0, :], mp[:])

            def lstm_gates_rb(rb, it):
                """gate matmuls + activations for block rb."""
                gact = []
                for g in range(4):
                    gp = ps.tile([H, rbsz], f32, tag="pb", bufs=2,
                                 name=f"gp_{it}_{rb}_{g}")
                    nc.tensor.matmul(gp[:], wgq[:, :, g * H:(g + 1) * H],
                                     gh_rb[rb][:], start=True, stop=True,
                                     perf_mode=DR)
                    ga = wk.tile([H, rbsz], bf16, tag=f"ga{g}", bufs=2,
                                 name=f"ga_{it}_{rb}_{g}")
                    nc.scalar.activation(
                        ga[:], gp[:],
                        AF.Tanh if g == 2 else AF.Sigmoid,
                        bias=bgc[:, g:g + 1])
                    gact.append(ga)
                return gact

            def lstm_cell_rb(rb, it, gact):
                """c/h elementwise update for block rb."""
                si, sf, tg, so = gact
                t1 = wk.tile([H, rbsz], bf16, tag="t1", bufs=2,
                             name=f"t1_{it}_{rb}")
                nc.vector.tensor_tensor(t1[:], sf[:], c_rb[rb][:], ALU.mult)
                t2 = wk.tile([H, rbsz], bf16, tag="t2", bufs=2,
                             name=f"t2_{it}_{rb}")
                nc.vector.tensor_tensor(t2[:], si[:], tg[:], ALU.mult)
                nc.vector.tensor_tensor(c_rb[rb][:], t1[:], t2[:], ALU.add)
                tnc = wk.tile([H, rbsz], bf16, tag="tnc", bufs=2,
                              name=f"tnc_{it}_{rb}")
                nc.scalar.activation(tnc[:], c_rb[rb][:], AF.Tanh)
                nc.vector.tensor_tensor(h_rb[rb][:], so[:], tnc[:], ALU.mult)
                nc.vector.tensor_copy(gh_rb[rb][:, 1, :], h_rb[rb][:])

            def ln_front_rb(rb, it):
                """transpose + stats + DVE-only rsqrt + affine apply."""
                trp = ps.tile([128, ntt, 128], bf16, tag="ptr", bufs=2,
                              name=f"trp_{it}_{rb}")
                mvb = wk.tile([128, ntt, 2], f32, tag="mvb", bufs=2,
                              name=f"mvb_{it}_{rb}")
                for t in range(ntt):
                    nc.tensor.transpose(
                        trp[:, t, :], h_rb[rb][:, t * 128:(t + 1) * 128],
                        ident[:])
                    stt = wk.tile([128, 6], f32, tag="st6", bufs=3,
                                  name=f"st_{it}_{rb}_{t}")
                    nc.vector.bn_stats(stt[:], trp[:, t, :])
                    nc.vector.bn_aggr(mvb[:, t, :], stt[:])
                # s = rsqrt(var+eps) via bitcast magic + 1 Newton (DVE only)
                vpe = wk.tile([128, ntt], f32, tag="vpe", bufs=2,
                              name=f"vpe_{it}_{rb}")
                nc.vector.tensor_scalar_add(vpe[:], mvb[:, :, 1], EPS)
                shi = wk.tile([128, ntt], i32, tag="shi", bufs=2,
                              name=f"shi_{it}_{rb}")
                nc.vector.tensor_scalar(shi[:], vpe[:].bitcast(i32), 1, None,
                                        op0=ALU.logical_shift_right)
                y0i = wk.tile([128, ntt], i32, tag="y0i", bufs=2,
                              name=f"y0i_{it}_{rb}")
                nc.vector.scalar_tensor_tensor(y0i[:], mgc[:], 0, shi[:],
                                               op0=ALU.bypass,
                                               op1=ALU.subtract)
                y = y0i[:].bitcast(f32)
                aa = wk.tile([128, ntt], f32, tag="aa", bufs=2,
                             name=f"aa_{it}_{rb}")
                bb = wk.tile([128, ntt], f32, tag="bb", bufs=2,
                             name=f"bb_{it}_{rb}")
                wt2 = wk.tile([128, ntt], f32, tag="wt2", bufs=2,
                              name=f"wt2_{it}_{rb}")
                sss = wk.tile([128, ntt], f32, tag="sss", bufs=2,
                              name=f"sss_{it}_{rb}")
                nc.vector.tensor_tensor(aa[:], y, y, ALU.mult)
                nc.vector.tensor_tensor(bb[:], vpe[:], aa[:], ALU.mult)
                nc.vector.tensor_scalar(wt2[:], bb[:], -0.5, 1.5,
                                        op0=ALU.mult, op1=ALU.add)
                nc.vector.tensor_tensor(sss[:], y, wt2[:], ALU.mult)
                hnr = wk.tile([128, ntt, 128], bf16, tag="hnr", bufs=2,
                              name=f"hnr_{it}_{rb}")
                for t in range(ntt):
                    nc.vector.tensor_scalar(hnr[:, t, :], trp[:, t, :],
                                            mvb[:, t, 0:1], sss[:, t:t + 1],
                                            op0=ALU.subtract, op1=ALU.mult)
                return hnr

            def ln_back_rb(rb, it, hnr):
                """transpose back -> hnL slice (fp8)."""
                hnp = ps.tile([128, ntt, 128], bf16, tag="ptr", bufs=2,
                              name=f"hnp_{it}_{rb}")
                for t in range(ntt):
                    nc.tensor.transpose(hnp[:, t, :], hnr[:, t, :], ident[:])
                dst = hnL[:, rb * rbsz:(rb + 1) * rbsz]
                if rb % 2 == 0:
                    nc.scalar.activation(dst, hnp[:], AF.Copy)
                else:
                    nc.vector.tensor_copy(dst, hnp[:])

            def mlp_stage_rb(rb, it):
                """local msg MLP on hnL block rb -> mloc + AG staging."""
                sl = slice(rb * rbsz, (rb + 1) * rbsz)
                m1p = ps.tile([H, rbsz], f32, tag="pb", bufs=2,
                              name=f"m1p_{it}_{rb}")
                nc.tensor.matmul(m1p[:], w1gt[:], hnL[:, sl],
                                 start=True, stop=True)
                m1s = wk.tile([H, rbsz], bf16, tag="m1s", bufs=2,
                              name=f"m1s_{it}_{rb}")
                nc.scalar.activation(m1s[:], m1p[:], AF.Relu, bias=b1c[:])
                m2p = ps.tile([H, rbsz], f32, tag="pb", bufs=2,
                              name=f"m2p_{it}_{rb}")
                nc.tensor.matmul(m2p[:], w2t[:], m1s[:], start=True, stop=True)
                m2s = wk.tile([H, rbsz], bf16, tag="m2s", bufs=2,
                              name=f"m2s_{it}_{rb}")
                nc.scalar.activation(m2s[:], m2p[:], AF.Relu, bias=b2c[:])
                mloc = st.tile([128, ntt, H], fp8, tag=f"mloc{rb}", bufs=2,
                               name=f"mloc_{it}_{rb}")
                m3p = ps.tile([H, rbsz], f32, tag="pb", bufs=2,
                              name=f"m3p_{it}_{rb}")
                for t in range(ntt):
                    nc.tensor.matmul(m3p[:, t * H:(t + 1) * H],
                                     m2s[:, t * 128:(t + 1) * 128],
                                     w3t[:], start=True, stop=True)
                nc.scalar.activation(mloc[:], m3p[:], AF.Copy)
                g = min(rb, 2) if rb >= 2 else 0
                g = [0, 0, 1, 2][rb]
                j = [0, 1, 0, 0][rb]
                jw = slice(j * ntt * H, (j + 1) * ntt * H)
                nc.sync.dma_start(cc_in[g][:, jw], mloc[:])
                return mloc

            def ag_start(g, it, nrbg):
                """AllGather both ranks' m; DMA the two rank blocks in."""
                w = nrbg * ntt * H
                nc.gpsimd.collective_compute(
                    "AllGather", ALU.bypass,
                    replica_groups=GROUPS,
                    ins=[cc_in[g][:].opt()], outs=[cc_out[g][:].opt()])
                t0 = wk.tile([128, w], fp8, tag=f"agb0_{g}", bufs=2,
                             name=f"agb0_{it}_{g}")
                t1 = wk.tile([128, w], fp8, tag=f"agb1_{g}", bufs=2,
                             name=f"agb1_{it}_{g}")
                nc.sync.dma_start(t0[:], cc_out[g][0:128, :])
                nc.sync.dma_start(t1[:], cc_out[g][128:256, :])
                return (t0, t1, w, nrbg)

            def ag_merge(g, it, st8):
                """partner block = copy rank0, predicated-overwrite rank1
                (mask = 1-rho, int32 over 4 packed fp8).  Deferred to the
                consuming iteration so it never blocks the DVE queue."""
                t0, t1, w, nrbg = st8
                mR = st.tile([128, nrbg * ntt, H], fp8, tag=f"mR{g}", bufs=2,
                             name=f"mR_{it}_{g}")
                nc.vector.tensor_copy(mR[:].bitcast(i32), t0[:].bitcast(i32))
                nc.vector.copy_predicated(mR[:].bitcast(i32),
                                          selm[:, :w // 4],
                                          t1[:].bitcast(i32))
                return mR

            # ================= vote (per rb) =================
            def vote_rb(rb):
                sl = slice(rb * rbsz, (rb + 1) * rbsz)
                v1p = ps.tile([H, rbsz], f32, tag="pb", bufs=2,
                              name=f"v1p_{rb}")
                nc.tensor.matmul(v1p[:], vw1gt[:], hnL[:, sl],
                                 start=True, stop=True)
                v1s = wk.tile([H, rbsz], bf16, tag="m1s", bufs=2,
                              name=f"v1s_{rb}")
                nc.scalar.activation(v1s[:], v1p[:], AF.Relu, bias=vb1c[:])
                v2p = ps.tile([H, rbsz], f32, tag="pb", bufs=2,
                              name=f"v2p_{rb}")
                nc.tensor.matmul(v2p[:], vw2t[:], v1s[:], start=True, stop=True)
                v2s = wk.tile([H, rbsz], bf16, tag="m2s", bufs=2,
                              name=f"v2s_{rb}")
                nc.scalar.activation(v2s[:], v2p[:], AF.Relu, bias=vb2c[:])
                v3t = ps.tile([H, rbsz], f32, tag="pb", bufs=2,
                              name=f"v3t_{rb}")
                nc.tensor.matmul(v3t[0:1, :], vw3t[:], v2s[:],
                                 start=True, stop=True)
                vos = wk.tile([1, rbsz], f32, tag="vos", bufs=2,
                              name=f"vos_{rb}")
                nc.scalar.activation(vos[:], v3t[0:1, :], AF.Copy)
                nc.sync.dma_start(votes_out[:, sl], vos[:])

            # ================= main loop =================
            # Software-pipelined across iterations: the next iteration's
            # own-pair matmuls are issued incrementally as each mloc block
            # is produced (fin_k), so the PE never drains during the
            # ACT/DVE back-half; the AG merge is deferred into the
            # consuming iteration so it never blocks the DVE queue.
            SLOTS = [("rem0", 0), ("rem1", 0), ("rem2", 0), ("drain", 0),
                     ("lstmA", 0),
                     ("rem0", 1), ("rem1", 1), ("rem2", 1), ("drain", 1),
                     ("lstmA", 1),
                     ("rem0", 2), ("rem1", 2), ("rem2", 2), ("drain", 2),
                     ("lstmA", 2), ("lstmB", 0),
                     ("rem0", 3), ("rem1", 3), ("rem2", 3), ("drain", 3),
                     ("lstmA", 3), ("lstmB", 1),
                     ("lnf", 0), ("lstmB", 2), ("lnf", 1), ("lnf", 2),
                     ("fin", 0), ("fin", 1),
                     ("lstmB", 3), ("lnf", 3), ("fin", 2), ("fin", 3)]
            mps = {}                  # accumulators for the CURRENT it
            ag_pend = [None, None, None]  # (t0, t1, w, nrbg) per group
            for it in range(2, iters + 1):
                if it == 2:           # prologue: everything from m1q
                    for rb in range(nrb):
                        mps[rb] = bmm_own_start(rb, it)
                    for k in range(nrb):
                        for rb in range(nrb):
                            bmm_own_frag(rb, mps[rb], it, k, None)
                    mRg = [None, None, None]
                else:                 # merge last iteration's gathers
                    mRg = [ag_merge(g, it, ag_pend[g]) for g in range(3)]
                nxt = {}
                lnst = {}
                gacts = {}
                mloc_new = [None] * nrb
                for ph, rb in SLOTS:
                    if ph == "rem0":
                        bmm_rem_rb(rb, mps[rb], it, mRg[0], 0, False)
                    elif ph == "rem1":
                        bmm_rem_rb(rb, mps[rb], it, mRg[1], 1, False)
                    elif ph == "rem2":
                        bmm_rem_rb(rb, mps[rb], it, mRg[2], 2, True)
                    elif ph == "drain":
                        drain_rb(rb, mps.pop(rb), it)
                    elif ph == "lstmA":
                        gacts[rb] = lstm_gates_rb(rb, it)
                    elif ph == "lstmB":
                        lstm_cell_rb(rb, it, gacts.pop(rb))
                    elif ph == "lnf":
                        lnst[rb] = ln_front_rb(rb, it)
                    else:
                        ln_back_rb(rb, it, lnst.pop(rb))
                        if it < iters:
                            k = rb
                            mloc_new[k] = mlp_stage_rb(k, it)
                            if k == 0:   # open next iteration's accs
                                for r2 in range(nrb):
                                    nxt[r2] = bmm_own_start(r2, it + 1)
                            for r2 in range(nrb):
                                bmm_own_frag(r2, nxt[r2], it + 1, k,
                                             mloc_new[k])
                            if k == 1:
                                ag_pend[0] = ag_start(0, it, 2)
                            elif k == 2:
                                ag_pend[1] = ag_start(1, it, 1)
                            elif k == 3:
                                ag_pend[2] = ag_start(2, it, 1)
                        else:
                            vote_rb(rb)
                mps = nxt

    nc.compile()
    return nc


_NC_CACHE = {}


def _get_module():
    key = (N, ITERS)
    if key not in _NC_CACHE:
        _NC_CACHE[key] = build_module(N, ITERS)
    return _NC_CACHE[key]


def _host_prep(inputs):
    """Fold weights, run init MLP + iteration 1 (rank-1), build in_maps."""
    g = lambda s: np.asarray(inputs[s], np.float32)
    x = g("x")
    k, n = g("k"), g("n")

    nk = np.stack([k, n], 1)
    a = np.maximum(nk @ g("init_w1").T + g("init_b1"), 0)
    a = np.maximum(a @ g("init_w2").T + g("init_b2"), 0)
    init0 = a @ g("init_w3").T + g("init_b3")          # [B, H]

    ln_g, ln_b = g("ln_g"), g("ln_b")

    def ln(h):
        mu = h.mean(-1, keepdims=True)
        xc = h - mu
        var = (xc * xc).mean(-1, keepdims=True)
        return xc / np.sqrt(var + EPS) * ln_g + ln_b

    def mlp_msg(e):
        t = np.maximum(e @ g("msg_w1").T + g("msg_b1"), 0)
        t = np.maximum(t @ g("msg_w2").T + g("msg_b2"), 0)
        return t @ g("msg_w3").T                       # NOTE: no +b3

    embed0 = ln(init0)                                 # [B, H]
    m0eff = mlp_msg(embed0) + g("msg_b3")              # [B, H] (with b3)

    # iteration 1 on host (rank-1 structure: msg1 = rowsums ⊗ m0eff)
    wih, whh = g("lstm_wih"), g("lstm_whh")
    bsum = g("lstm_bih") + g("lstm_bhh")               # [4H]
    u = m0eff @ wih.T                                  # [B, 4H]
    v = init0 @ whh.T + bsum                           # [B, 4H]
    sig = lambda z: 1.0 / (1.0 + np.exp(-z))

    com = {
        "w1gt": (g("msg_w1") * ln_g[None, :]).T.astype(BF),
        "w2t": g("msg_w2").T.astype(BF),
        "w3t": g("msg_w3").T.astype(BF),
        "vw1gt": (g("vote_w1") * ln_g[None, :]).T.astype(BF),
        "vw2t": g("vote_w2").T.astype(BF),
        "vw3t": g("vote_w3").T.astype(BF),              # [H, 1]
        "wgq": np.stack([wih.T, whh.T], axis=1).astype(E4),
        "b1c": (g("msg_w1") @ ln_b + g("msg_b1")).reshape(H, 1).astype(np.float32),
        "b2c": g("msg_b2").reshape(H, 1).astype(np.float32),
        "vb1c": (g("vote_w1") @ ln_b + g("vote_b1")).reshape(H, 1).astype(np.float32),
        "vb2c": g("vote_b2").reshape(H, 1).astype(np.float32),
        "bgc": bsum.reshape(4, H).T.astype(np.float32).copy(),
        "b3r": g("msg_b3").reshape(1, H).astype(BF),
        "ident": np.eye(H, dtype=BF),
    }

    # per-batch: fp8-consistent x, row sums, host iteration 1, full m1
    xq8, rs_full, h1_full, c1_full, m1_full = {}, {}, {}, {}, {}
    for b in range(B):
        xq8[b] = x[b].astype(E4)                       # [N, N] fp8
        rs_full[b] = xq8[b].astype(np.float32).sum(1)  # [N]
        gates1 = rs_full[b][:, None] * u[b][None, :] + v[b][None, :]
        i1, f1, g1, o1 = np.split(gates1, 4, 1)
        c1f = sig(i1) * np.tanh(g1)                    # [N, H]
        h1f = sig(o1) * np.tanh(c1f)                   # [N, H]
        c1_full[b], h1_full[b] = c1f, h1f
        m1_full[b] = mlp_msg(ln(h1f)).astype(np.float32)  # [N, H] (no b3)

    kc = N // 128
    in_maps = []
    for core in range(NCORES):
        b = core // 2
        rho = core % 2
        r0 = rho * R
        perm = np.r_[r0:r0 + R, R - r0:N - r0]         # [own | partner] rows
        m = dict(com)
        # x^T with contraction rows permuted to [own 2048 | partner 2048]
        m["xt"] = np.ascontiguousarray(xq8[b].T[perm, r0:r0 + R])
        m["rs"] = rs_full[b][r0:r0 + R].reshape(1, R).astype(BF)
        m["h1"] = np.ascontiguousarray(h1_full[b][r0:r0 + R].T).astype(BF)
        m["c1"] = np.ascontiguousarray(c1_full[b][r0:r0 + R].T).astype(BF)
        m["m1q"] = np.ascontiguousarray(
            m1_full[b][perm].reshape(kc, 128, H).transpose(1, 0, 2)).astype(E4)
        m["selm"] = np.full((H, 2 * (N // 8) // 128 * H // 4), 1 - rho,
                            np.int32)
        in_maps.append(m)
    return in_maps


def kernel(**inputs):
    nc = _get_module()
    in_maps = _host_prep(inputs)
    res = run_bass_kernel_spmd(nc, in_maps, core_ids=list(range(NCORES)))
    mask = np.asarray(inputs["mask"], np.float64)
    vb3 = float(np.asarray(inputs["vote_b3"], np.float64).reshape(-1)[0])
    out = np.zeros(B, np.float32)
    for b in range(B):
        votes = np.concatenate([
            res.results[2 * b]["votes"].reshape(-1),
            res.results[2 * b + 1]["votes"].reshape(-1),
        ]).astype(np.float64) + vb3
        s = float((votes * mask[b]).sum())
        out[b] = 1.0 / (1.0 + np.exp(-s))
    return out
